# revision 9
# baseline (speedup 1.0000x reference)
"""EvoBlock Trainium2 kernel: 8-core SPMD, row-sharded (sequence parallel).

Layout strategy:
  - seq side: q-rows sharded (48 rows/core), K/V computed for all rows on
    every core (seq is tiny). PairToSequence bias computed from own pair rows.
  - pair side: channels-on-partitions [c, row, col]; convs as per-output-row
    shifted matmuls accumulating taps in PSUM. Halos come from host-provided
    padded shards (90 rows each side); out-of-image rows are zero-masked.
  - collectives: AllGather of s (SequenceToPair projection, 25KB/core) +
    2 AllReduces of BatchNorm partial stats (1KB).
"""
import sys

sys.path.insert(0, "/opt/trn_rl_repo")

from contextlib import ExitStack

import numpy as np
import ml_dtypes

import concourse.bacc as bacc
import concourse.mybir as mybir
import concourse.tile as tile
from concourse.bass_utils import run_bass_kernel_spmd

F32 = mybir.dt.float32
BF16 = mybir.dt.bfloat16
AF = mybir.ActivationFunctionType
ALU = mybir.AluOpType

B, L, CS, CZ, H = 1, 384, 384, 128, 8
HW = CS // H  # 48
NC = 8
OWN = L // NC  # 48 rows per core
HALO = 90  # conv-chain halo depth in rows
R = OWN + 2 * HALO  # 228 local rows in x_pre space
EPS = 1e-5
NPIX = L * L  # global BN pixel count
OWNPIX = OWN * L  # 18432

# local-row index ranges (x_pre local index l in [0, R))
L_B0C1_LO, L_B0C1_HI = HALO - 42, HALO + OWN + 42  # b0c1 out rows [48, 180)
L_X1_LO, L_X1_HI = HALO - 41, HALO + OWN + 41  # x1 rows [49, 179)
L_B1C1_LO, L_B1C1_HI = HALO - 5, HALO + OWN + 5  # b1c1 out rows [85, 143)
L_X2_LO, L_X2_HI = HALO - 3, HALO + OWN + 3  # x2 rows [87, 141)
L_OWN_LO, L_OWN_HI = HALO, HALO + OWN  # [90, 138)

N_B0C1 = L_B0C1_HI - L_B0C1_LO  # 132
N_X1 = L_X1_HI - L_X1_LO  # 130
N_B1C1 = L_B1C1_HI - L_B1C1_LO  # 58
N_X2 = L_X2_HI - L_X2_LO  # 54

_CACHE = {}


def _taps(kk):
    """Tap order for a kk x kk conv: center first (start=True, full width)."""
    c = kk // 2
    order = [(c, c)]
    for ky in range(kk):
        for kx in range(kk):
            if (ky, kx) != (c, c):
                order.append((ky, kx))
    return order


def _conv_rows(nc, win_pool, psum_pool, cfg):
    """One conv stage: stream input rows into a ring of window tiles, then
    per output row accumulate all taps (shifted matmuls) in PSUM."""
    wt = cfg["wt"]  # SBUF weights [128, T, 128] bf16, tap-major (_taps order)
    taps = cfg["taps"]
    dil = cfg["dil"]
    load_row = cfg["load_row"]  # f(r, tag) -> window tile for input row r
    out_lo, out_hi = cfg["out_range"]
    in_n = cfg["in_n"]
    off = cfg["off"]  # input-local index = out_row + dy*dil - off
    post = cfg["post"]  # f(l, psum)
    nwin = cfg["nwin"]
    kk = int(len(taps) ** 0.5)
    c = kk // 2

    rows = {}

    def ensure(r):
        if r not in rows:
            rows[r] = load_row(r, f"w{r % nwin}")
        return rows[r]

    for l in range(out_lo, out_hi):
        psum = psum_pool.tile([128, L], F32, tag="convp")
        for t_i, (ky, kx) in enumerate(taps):
            dy, dx = (ky - c) * dil, (kx - c) * dil
            wtile = ensure(l + dy - off)
            a = max(0, -dx)
            b = L - max(0, dx)
            nc.tensor.matmul(
                psum[:, a:b],
                wt[:, t_i, :],
                wtile[:, a + dx : b + dx],
                start=(t_i == 0),
                stop=(t_i == len(taps) - 1),
            )
        post(l, psum)


def _build_program():
    nc = bacc.Bacc()

    def din(name, shape, dt=BF16):
        return nc.declare_dram_parameter(name, list(shape), dt, isOutput=False)

    pair_cT = din("pair_cT", (CZ, R, L), BF16)
    pair_px = din("pair_px", (OWNPIX, CZ), F32)
    seq_full = din("seq_full", (L, CS), F32)
    seq_own = din("seq_own", (OWN, CS), F32)
    mask_in = din("mask_in", (128, R), F32)
    sel_in = din("sel_in", (L, R), BF16)
    ident_in = din("ident_in", (128, 128), BF16)

    wq_in = din("wq", (CS, CS))
    wk_in = din("wk", (CS, CS))
    wv_in = din("wv", (CS, CS))
    wg_in = din("wg", (CS, CS))
    wo_in = din("wo", (CS, CS))
    w1s_in = din("w1s", (CS, 4 * CS))
    w2s_in = din("w2s", (4 * CS, CS))
    wsp_in = din("wsp", (CS, CZ))
    wso_in = din("wso", (CZ, CZ))
    wp2s_in = din("wp2s", (CZ, H))
    w1p_in = din("w1p", (CZ, 4 * CZ))
    w2p_in = din("w2p", (4 * CZ, CZ))
    cw0a_in = din("cw0a", (9, CZ, CZ))
    cw0b_in = din("cw0b", (9, CZ, CZ))
    cw1a_in = din("cw1a", (9, CZ, CZ))
    cw1b_in = din("cw1b", (9, CZ, CZ))
    cwo_in = din("cwo", (49, CZ, CZ))

    bq_in = din("bq", (HW, H), F32)
    bk_in = din("bk", (HW, H), F32)
    bv_in = din("bv", (1, CS), BF16)
    bg_in = din("bg", (1, CS), BF16)
    bo_in = din("bo", (1, CS), BF16)
    b1s_in = din("b1s", (1, 4 * CS), BF16)
    b2s_in = din("b2s", (1, CS), BF16)
    bsp_in = din("bsp", (1, CZ), BF16)
    bso_in = din("bso", (CZ, 1), F32)
    bp2s_in = din("bp2s", (1, H), BF16)
    b1p_in = din("b1p", (1, 4 * CZ), BF16)
    ncs1_in = din("ncs1", (1, 4 * CZ), BF16)
    b2p_in = din("b2p", (CZ, 1), F32)
    cob_in = din("cob", (CZ, 1), F32)
    bn0g_in = din("bn0g", (CZ, 1), F32)
    bn0b_in = din("bn0b", (CZ, 1), F32)
    bn1g_in = din("bn1g", (CZ, 1), F32)
    bn1b_in = din("bn1b", (CZ, 1), F32)

    seq_out = nc.declare_dram_parameter("seq_out", [OWN, CS], F32, isOutput=True)
    pair_out = nc.declare_dram_parameter("pair_out", [CZ, OWN, L], F32, isOutput=True)

    with ExitStack() as ctx:
        tc = ctx.enter_context(tile.TileContext(nc))

        dram = ctx.enter_context(tc.tile_pool(name="dram", bufs=1, space="DRAM"))
        xpre_d = dram.tile([CZ, R, L], BF16, tag="xpre")
        b0c1r_d = dram.tile([CZ, N_B0C1, L], BF16, tag="b0c1r")
        x1_d = dram.tile([CZ, N_X1, L], BF16, tag="x1")
        b1c1r_d = dram.tile([CZ, N_B1C1, L], BF16, tag="b1c1r")
        x2_d = dram.tile([CZ, N_X2, L], BF16, tag="x2")
        ccg_in = dram.tile([OWN, CZ], F32, tag="ccgi")
        ccg_out = dram.tile([L, CZ], F32, tag="ccgo", addr_space="Shared")
        ccr0_in = dram.tile([CZ, 2], F32, tag="ccr0i")
        ccr0_out = dram.tile([CZ, 2], F32, tag="ccr0o", addr_space="Shared")
        ccr1_in = dram.tile([CZ, 2], F32, tag="ccr1i")
        ccr1_out = dram.tile([CZ, 2], F32, tag="ccr1o", addr_space="Shared")

        cpool = ctx.enter_context(tc.tile_pool(name="const", bufs=1))

        mask = cpool.tile([128, R], F32, tag="mask")
        nc.sync.dma_start(out=mask[:], in_=mask_in[:])
        ident = cpool.tile([128, 128], BF16, tag="ident")
        nc.sync.dma_start(out=ident[:], in_=ident_in[:])
        ones_row = cpool.tile([1, 512], BF16, tag="ones")
        nc.vector.memset(ones_row[:], 1.0)
        eps128 = cpool.tile([128, 1], F32, tag="eps")
        nc.vector.memset(eps128[:], EPS)

        qcT = cpool.tile([64, L], BF16, tag="qcT")
        klT = cpool.tile([64, R], F32, tag="klT")
        s0tab = cpool.tile([128, R], F32, tag="s0tab")
        t0tab = cpool.tile([128, R], F32, tag="t0tab")
        s1tab = cpool.tile([128, N_X1], F32, tag="s1tab")
        t1tab = cpool.tile([128, N_X1], F32, tag="t1tab")

        def transpose_to(dst_ap, src_ap, pspool, kpart, mpart):
            """dst[mpart, kpart] = src[kpart, mpart].T via PE, ACT copy out."""
            pt = pspool.tile([mpart, kpart], src_ap.dtype, tag="trp", bufs=2)
            nc.tensor.transpose(pt[:], src_ap, ident[:kpart, :kpart])
            nc.scalar.copy(dst_ap, pt[:])

        # ============ Phase P: p2s bias (pair LN -> 8-dim proj) ============
        bias_h = []
        with tc.tile_pool(name="p2s", bufs=3) as pp, \
             tc.tile_pool(name="p2sb", bufs=1) as pb:
          with tc.tile_pool(name="p2spp", bufs=2, space="PSUM") as ppp:
            wp2s = pb.tile([CZ, H], BF16, tag="wp2s")
            nc.sync.dma_start(out=wp2s[:], in_=wp2s_in[:])
            bp2s = pb.tile([1, H], BF16, tag="bp2s")
            nc.sync.dma_start(out=bp2s[:], in_=bp2s_in[:])
            bias_sb = pb.tile([H, OWNPIX], BF16, tag="bias_sb")
            pxr = pair_px.rearrange("(t p) c -> t p c", p=128)
            for q in range(OWNPIX // 512):
                xq = pp.tile([CZ, 512], BF16, tag="xq")
                for t4 in range(4):
                    t = 4 * q + t4
                    xt = pp.tile([128, CZ], F32, tag="px")
                    nc.sync.dma_start(out=xt[:], in_=pxr[t])
                    mv = pp.tile([128, 6], F32, tag="mv")
                    nc.vector.bn_stats(out=mv[:], in_=xt[:])
                    st = pp.tile([128, 2], F32, tag="st")
                    nc.vector.bn_aggr(out=st[:], in_=mv[:])
                    rstd = pp.tile([128, 1], F32, tag="rstd")
                    nc.scalar.activation(
                        out=rstd[:], in_=st[:, 1:2], func=AF.Sqrt, bias=eps128[:]
                    )
                    nc.vector.reciprocal(rstd[:], rstd[:])
                    xb = pp.tile([128, CZ], BF16, tag="xb")
                    nc.vector.tensor_scalar(
                        xb[:], xt[:], st[:, 0:1], rstd[:], ALU.subtract, ALU.mult
                    )
                    pt = ppp.tile([128, 128], BF16, tag="trp")
                    nc.tensor.transpose(pt[:], xb[:], ident[:])
                    nc.vector.tensor_copy(xq[:, 128 * t4 : 128 * t4 + 128], pt[:])
                pbh = ppp.tile([H, 512], F32, tag="pb")
                nc.tensor.matmul(pbh[:], wp2s[:], xq[:], start=True, stop=False)
                nc.tensor.matmul(pbh[:], bp2s[:], ones_row[:], start=False, stop=True)
                nc.scalar.copy(bias_sb[:, 512 * q : 512 * q + 512], pbh[:])
          if True:
            for h in range(H):
                bh = pb.tile([OWN, L], BF16, tag=f"biash{h}")
                nc.sync.dma_start(out=bh[:], in_=bias_sb[h : h + 1, :])
                bias_h.append(bh)

            # ============ Phase S: seq pipeline (q-sharded attention) ======
            with tc.tile_pool(name="seqw", bufs=1) as sw, \
                 tc.tile_pool(name="seqt", bufs=2) as stp, \
                 tc.tile_pool(name="seqps", bufs=4, space="PSUM") as sps:
                wq = sw.tile([128, 3, CS], BF16, tag="wq")
                wk = sw.tile([128, 3, CS], BF16, tag="wk")
                wv = sw.tile([128, 3, CS], BF16, tag="wv")
                wg = sw.tile([128, 3, CS], BF16, tag="wg")
                wo = sw.tile([128, 3, CS], BF16, tag="wo")
                for w, wi in ((wq, wq_in), (wk, wk_in), (wv, wv_in),
                              (wg, wg_in), (wo, wo_in)):
                    nc.sync.dma_start(
                        out=w[:], in_=wi.rearrange("(a p) n -> p a n", p=128)
                    )
                bqs = sw.tile([HW, H], F32, tag="bqs")
                nc.sync.dma_start(out=bqs[:], in_=bq_in[:])
                bks = sw.tile([HW, H], F32, tag="bks")
                nc.sync.dma_start(out=bks[:], in_=bk_in[:])
                bvr = sw.tile([1, CS], BF16, tag="bvr")
                nc.sync.dma_start(out=bvr[:], in_=bv_in[:])
                bgr = sw.tile([1, CS], BF16, tag="bgr")
                nc.sync.dma_start(out=bgr[:], in_=bg_in[:])
                bor = sw.tile([1, CS], BF16, tag="bor")
                nc.sync.dma_start(out=bor[:], in_=bo_in[:])

                def ln_tile(pool, src_ap, npart, tag):
                    """LayerNorm (g/b folded into weights): (x-m)*rstd -> bf16."""
                    mv_ = pool.tile([npart, 6], F32, tag=tag + "mv")
                    nc.vector.bn_stats(out=mv_[:], in_=src_ap)
                    ag_ = pool.tile([npart, 2], F32, tag=tag + "ag")
                    nc.vector.bn_aggr(out=ag_[:], in_=mv_[:])
                    rs_ = pool.tile([npart, 1], F32, tag=tag + "rs")
                    nc.scalar.activation(
                        out=rs_[:], in_=ag_[:, 1:2], func=AF.Sqrt,
                        bias=eps128[:npart, :],
                    )
                    nc.vector.reciprocal(rs_[:], rs_[:])
                    xh_ = pool.tile([npart, CS], BF16, tag=tag + "xh")
                    nc.vector.tensor_scalar(
                        xh_[:], src_ap, ag_[:, 0:1], rs_[:], ALU.subtract, ALU.mult
                    )
                    return xh_

                # LN(seq) full rows -> xhatT [128, 3(c-chunk), 384rows]
                xhatT = sw.tile([128, 3, L], BF16, tag="xhatT")
                sqr = seq_full.rearrange("(a p) n -> a p n", p=128)
                for a in range(3):
                    st_ = stp.tile([128, CS], F32, tag="seqtile")
                    nc.sync.dma_start(out=st_[:], in_=sqr[a])
                    xh = ln_tile(stp, st_[:], 128, "lf")
                    for cc_ in range(3):
                        transpose_to(
                            xhatT[:, cc_, 128 * a : 128 * a + 128],
                            xh[:, 128 * cc_ : 128 * cc_ + 128],
                            sps, 128, 128,
                        )
                # LN(seq_own) -> xhatT_own [128, 3, 48]
                so = stp.tile([OWN, CS], F32, tag="seqown")
                nc.sync.dma_start(out=so[:], in_=seq_own[:])
                xho = ln_tile(stp, so[:], OWN, "lo")
                xhoT = sw.tile([128, 3, OWN], BF16, tag="xhoT")
                for cc_ in range(3):
                    transpose_to(
                        xhoT[:, cc_, :], xho[:, 128 * cc_ : 128 * cc_ + 128],
                        sps, OWN, 128,
                    )

                # per-head projections
                kT_h, qT_h, v_h = [], [], []
                for h in range(H):
                    hs = slice(HW * h, HW * h + HW)
                    pk = sps.tile([HW, L], F32, tag="sp")
                    for cc_ in range(3):
                        nc.tensor.matmul(
                            pk[:], wk[:, cc_, hs], xhatT[:, cc_, :],
                            start=(cc_ == 0), stop=(cc_ == 2),
                        )
                    kt = sw.tile([HW, L], BF16, tag=f"kT{h}")
                    nc.scalar.activation(
                        out=kt[:], in_=pk[:], func=AF.Identity,
                        bias=bks[:, h : h + 1],
                    )
                    kT_h.append(kt)

                    pq = sps.tile([HW, OWN], F32, tag="sp")
                    for cc_ in range(3):
                        nc.tensor.matmul(
                            pq[:], wq[:, cc_, hs], xhoT[:, cc_, :],
                            start=(cc_ == 0), stop=(cc_ == 2),
                        )
                    qt = sw.tile([HW, OWN], BF16, tag=f"qT{h}")
                    nc.scalar.activation(
                        out=qt[:], in_=pq[:], func=AF.Identity,
                        bias=bqs[:, h : h + 1],
                    )
                    qT_h.append(qt)

                    vh = sw.tile([128, 3, HW], BF16, tag=f"v{h}")
                    for rc in range(3):
                        pv = sps.tile([128, HW], F32, tag="sp")
                        for cc_ in range(3):
                            nc.tensor.matmul(
                                pv[:],
                                xhatT[:, cc_, 128 * rc : 128 * rc + 128],
                                wv[:, cc_, hs],
                                start=(cc_ == 0), stop=False,
                            )
                        nc.tensor.matmul(
                            pv[:], ones_row[:, :128], bvr[:, hs],
                            start=False, stop=True,
                        )
                        nc.scalar.copy(vh[:, rc, :], pv[:])
                    v_h.append(vh)

                # gate
                pg = sps.tile([OWN, CS], F32, tag="sp")
                for cc_ in range(3):
                    nc.tensor.matmul(
                        pg[:], xhoT[:, cc_, :], wg[:, cc_, :],
                        start=(cc_ == 0), stop=False,
                    )
                nc.tensor.matmul(
                    pg[:], ones_row[:, :OWN], bgr[:], start=False, stop=True
                )
                g_own = stp.tile([OWN, CS], BF16, tag="gown")
                nc.scalar.activation(out=g_own[:], in_=pg[:], func=AF.Sigmoid)

                # attention per head
                o_sb = stp.tile([OWN, CS], BF16, tag="osb")
                for h in range(H):
                    ps = sps.tile([OWN, L], F32, tag="sp")
                    nc.tensor.matmul(
                        ps[:], qT_h[h][:], kT_h[h][:], start=True, stop=True
                    )
                    ssb = stp.tile([OWN, L], F32, tag="ssb")
                    nc.vector.tensor_tensor(ssb[:], ps[:], bias_h[h][:], ALU.add)
                    mx = stp.tile([OWN, 1], F32, tag="mx")
                    nc.vector.reduce_max(mx[:], ssb[:], mybir.AxisListType.X)
                    nc.vector.tensor_scalar_mul(mx[:], mx[:], -1.0)
                    aexp = stp.tile([OWN, L], BF16, tag="aexp")
                    sume = stp.tile([OWN, 1], F32, tag="sume")
                    nc.scalar.activation(
                        out=aexp[:], in_=ssb[:], func=AF.Exp, bias=mx[:],
                        accum_out=sume[:],
                    )
                    nc.vector.reciprocal(sume[:], sume[:])
                    po = sps.tile([OWN, HW], F32, tag="sp")
                    for rc in range(3):
                        at = stp.tile([128, OWN], BF16, tag="at")
                        transpose_to(
                            at[:], aexp[:, 128 * rc : 128 * rc + 128],
                            sps, OWN, 128,
                        )
                        nc.tensor.matmul(
                            po[:], at[:], v_h[h][:, rc, :],
                            start=(rc == 0), stop=(rc == 2),
                        )
                    nc.vector.tensor_scalar_mul(
                        o_sb[:, HW * h : HW * h + HW], po[:], sume[:]
                    )

                # gated output projection + residual
                go = stp.tile([OWN, CS], BF16, tag="go")
                nc.vector.tensor_mul(go[:], g_own[:], o_sb[:])
                goT = stp.tile([128, 3, OWN], BF16, tag="goT")
                for cc_ in range(3):
                    transpose_to(
                        goT[:, cc_, :], go[:, 128 * cc_ : 128 * cc_ + 128],
                        sps, OWN, 128,
                    )
                pz = sps.tile([OWN, CS], F32, tag="sp")
                for cc_ in range(3):
                    nc.tensor.matmul(
                        pz[:], goT[:, cc_, :], wo[:, cc_, :],
                        start=(cc_ == 0), stop=False,
                    )
                nc.tensor.matmul(
                    pz[:], ones_row[:, :OWN], bor[:], start=False, stop=True
                )
                seq1 = stp.tile([OWN, CS], F32, tag="seq1")
                nc.vector.tensor_tensor(seq1[:], pz[:], so[:], ALU.add)

                # mlps
                w1s = sw.tile([128, 3, 4 * CS], BF16, tag="w1s")
                nc.sync.dma_start(
                    out=w1s[:], in_=w1s_in.rearrange("(a p) n -> p a n", p=128)
                )
                w2s = sw.tile([128, 12, CS], BF16, tag="w2s")
                nc.sync.dma_start(
                    out=w2s[:], in_=w2s_in.rearrange("(a p) n -> p a n", p=128)
                )
                b1sr = sw.tile([1, 4 * CS], BF16, tag="b1sr")
                nc.sync.dma_start(out=b1sr[:], in_=b1s_in[:])
                b2sr = sw.tile([1, CS], BF16, tag="b2sr")
                nc.sync.dma_start(out=b2sr[:], in_=b2s_in[:])

                xh2 = ln_tile(stp, seq1[:], OWN, "l2")
                xh2T = stp.tile([128, 3, OWN], BF16, tag="xh2T")
                for cc_ in range(3):
                    transpose_to(
                        xh2T[:, cc_, :], xh2[:, 128 * cc_ : 128 * cc_ + 128],
                        sps, OWN, 128,
                    )
                h1r = stp.tile([OWN, 4 * CS], BF16, tag="h1r")
                for nch in range(3):
                    ph = sps.tile([OWN, 512], F32, tag="sp")
                    nsl = slice(512 * nch, 512 * nch + 512)
                    for cc_ in range(3):
                        nc.tensor.matmul(
                            ph[:], xh2T[:, cc_, :], w1s[:, cc_, nsl],
                            start=(cc_ == 0), stop=False,
                        )
                    nc.tensor.matmul(
                        ph[:], ones_row[:, :OWN], b1sr[:, nsl],
                        start=False, stop=True,
                    )
                    nc.scalar.activation(out=h1r[:, nsl], in_=ph[:], func=AF.Relu)
                h1rT = stp.tile([128, 12, OWN], BF16, tag="h1rT")
                for kc in range(12):
                    transpose_to(
                        h1rT[:, kc, :], h1r[:, 128 * kc : 128 * kc + 128],
                        sps, OWN, 128,
                    )
                pm = sps.tile([OWN, CS], F32, tag="sp")
                for kc in range(12):
                    nc.tensor.matmul(
                        pm[:], h1rT[:, kc, :], w2s[:, kc, :],
                        start=(kc == 0), stop=False,
                    )
                nc.tensor.matmul(
                    pm[:], ones_row[:, :OWN], b2sr[:], start=False, stop=True
                )
                seq2 = stp.tile([OWN, CS], F32, tag="seq2")
                nc.vector.tensor_tensor(seq2[:], pm[:], seq1[:], ALU.add)
                nc.sync.dma_start(out=seq_out[:], in_=seq2[:])

                # s2p projection s = LN(seq2) @ wsp + bsp, then AllGather
                wsp = sw.tile([128, 3, CZ], BF16, tag="wsp")
                nc.sync.dma_start(
                    out=wsp[:], in_=wsp_in.rearrange("(a p) n -> p a n", p=128)
                )
                bspr = sw.tile([1, CZ], BF16, tag="bspr")
                nc.sync.dma_start(out=bspr[:], in_=bsp_in[:])
                xh3 = ln_tile(stp, seq2[:], OWN, "l3")
                xh3T = stp.tile([128, 3, OWN], BF16, tag="xh3T")
                for cc_ in range(3):
                    transpose_to(
                        xh3T[:, cc_, :], xh3[:, 128 * cc_ : 128 * cc_ + 128],
                        sps, OWN, 128,
                    )
                psp = sps.tile([OWN, CZ], F32, tag="sp")
                for cc_ in range(3):
                    nc.tensor.matmul(
                        psp[:], xh3T[:, cc_, :], wsp[:, cc_, :],
                        start=(cc_ == 0), stop=False,
                    )
                nc.tensor.matmul(
                    psp[:], ones_row[:, :OWN], bspr[:], start=False, stop=True
                )
                s_own = stp.tile([OWN, CZ], F32, tag="sown")
                nc.scalar.copy(s_own[:], psp[:])
                nc.sync.dma_start(out=ccg_in[:], in_=s_own[:])

        nc.gpsimd.collective_compute(
            "AllGather", ALU.bypass,
            ins=[ccg_in.opt()], outs=[ccg_out.opt()],
            replica_groups=[list(range(NC))],
        )

        # ============ Phase G: qcT [64, 384] and klT [64, 228] ============
        with tc.tile_pool(name="gph", bufs=2) as gp, \
             tc.tile_pool(name="gps", bufs=2, space="PSUM") as gps:
            sel_sb = gp.tile([128, 3, R], BF16, tag="sel")
            nc.sync.dma_start(
                out=sel_sb[:], in_=sel_in.rearrange("(a p) r -> p a r", p=128)
            )
            pkl = gps.tile([64, R], F32, tag="pkl")
            for a in range(3):
                srow = gp.tile([128, CZ], F32, tag="srow")
                nc.sync.dma_start(
                    out=srow[:],
                    in_=ccg_out.rearrange("(a p) c -> a p c", p=128)[a],
                )
                srb = gp.tile([128, CZ], BF16, tag="srb")
                nc.vector.tensor_copy(srb[:], srow[:])
                pq2 = gps.tile([64, 128], BF16, tag="pq2")
                nc.tensor.transpose(pq2[:], srb[:, 0:64], ident[:])
                nc.scalar.copy(qcT[:, 128 * a : 128 * a + 128], pq2[:])
                nc.tensor.matmul(
                    pkl[:], srb[:, 64:128], sel_sb[:, a, :],
                    start=(a == 0), stop=(a == 2),
                )
            nc.scalar.copy(klT[:], pkl[:])

        # ============ Phase X: x_pre = pair + s2p update; bn0 stats =======
        with tc.tile_pool(name="xph", bufs=4) as xp, \
             tc.tile_pool(name="xpb", bufs=1) as xpb, \
             tc.tile_pool(name="xps", bufs=4, space="PSUM") as xps:
            wso = xpb.tile([CZ, CZ], BF16, tag="wso")
            nc.sync.dma_start(out=wso[:], in_=wso_in[:])
            bso = xpb.tile([CZ, 1], F32, tag="bso")
            nc.sync.dma_start(out=bso[:], in_=bso_in[:])
            bn0s = xpb.tile([128, R], F32, tag="bn0s")
            bn0q = xpb.tile([128, OWN], F32, tag="bn0q")
            for r in range(R):
                cc_t = xp.tile([128, L], BF16, tag="cc")
                nc.vector.tensor_scalar_mul(cc_t[0:64, :], qcT[:], klT[:, r : r + 1])
                nc.vector.tensor_scalar_sub(cc_t[64:128, :], qcT[:], klT[:, r : r + 1])
                pxm = xps.tile([128, L], F32, tag="pxm")
                nc.tensor.matmul(pxm[:], wso[:], cc_t[:], start=True, stop=True)
                prow = xp.tile([128, L], BF16, tag="prow")
                nc.sync.dma_start(out=prow[:], in_=pair_cT[:, r, :])
                xrow = xp.tile([128, L], BF16, tag="xrow")
                nc.vector.scalar_tensor_tensor(
                    out=xrow[:], in0=pxm[:], scalar=bso[:], in1=prow[:],
                    op0=ALU.add, op1=ALU.add,
                    accum_out=bn0s[:, r : r + 1],
                )
                nc.sync.dma_start(out=xpre_d[:, r, :], in_=xrow[:])
                if L_OWN_LO <= r < L_OWN_HI:
                    junk = xp.tile([128, L], BF16, tag="junk")
                    nc.scalar.activation(
                        out=junk[:], in_=xrow[:], func=AF.Square,
                        accum_out=bn0q[:, r - L_OWN_LO : r - L_OWN_LO + 1],
                    )
            part0 = xpb.tile([CZ, 2], F32, tag="part0")
            nc.vector.reduce_sum(
                part0[:, 0:1], bn0s[:, L_OWN_LO:L_OWN_HI], mybir.AxisListType.X
            )
            nc.vector.reduce_sum(part0[:, 1:2], bn0q[:], mybir.AxisListType.X)
            nc.sync.dma_start(out=ccr0_in[:], in_=part0[:])

        nc.gpsimd.collective_compute(
            "AllReduce", ALU.add,
            ins=[ccr0_in.opt()], outs=[ccr0_out.opt()],
            replica_groups=[list(range(NC))],
        )

        def bn_fold(cc_out_t, g_in, b_in, stab, ttab, msk_lo, msk_hi):
            """From allreduced [c,2] sums build masked bn scale/shift tables:
            stab[:, i] = g*rstd*mask, ttab[:, i] = (b - mean*g*rstd)*mask."""
            with tc.tile_pool(name="bnf", bufs=1) as bf:
                gl = bf.tile([CZ, 2], F32, tag="gl")
                nc.sync.dma_start(out=gl[:], in_=cc_out_t[:])
                gt = bf.tile([CZ, 1], F32, tag="gt")
                nc.sync.dma_start(out=gt[:], in_=g_in[:])
                bt = bf.tile([CZ, 1], F32, tag="bt")
                nc.sync.dma_start(out=bt[:], in_=b_in[:])
                mo = bf.tile([CZ, 2], F32, tag="mo")
                nc.vector.tensor_scalar_mul(mo[:], gl[:], 1.0 / NPIX)
                var = bf.tile([CZ, 1], F32, tag="var")
                nc.vector.tensor_mul(var[:], mo[:, 0:1], mo[:, 0:1])
                nc.vector.tensor_sub(var[:], mo[:, 1:2], var[:])
                rs_ = bf.tile([CZ, 1], F32, tag="rs")
                nc.scalar.activation(
                    out=rs_[:], in_=var[:], func=AF.Sqrt, bias=eps128[:]
                )
                nc.vector.reciprocal(rs_[:], rs_[:])
                sc = bf.tile([CZ, 1], F32, tag="sc")
                nc.vector.tensor_mul(sc[:], rs_[:], gt[:])
                sh = bf.tile([CZ, 1], F32, tag="sh")
                nc.vector.tensor_mul(sh[:], mo[:, 0:1], sc[:])
                nc.vector.tensor_sub(sh[:], bt[:], sh[:])
                n = msk_hi - msk_lo
                nc.vector.tensor_tensor(
                    stab[:], sc[:].to_broadcast([CZ, n]),
                    mask[:, msk_lo:msk_hi], ALU.mult,
                )
                nc.vector.tensor_tensor(
                    ttab[:], sh[:].to_broadcast([CZ, n]),
                    mask[:, msk_lo:msk_hi], ALU.mult,
                )

        bn_fold(ccr0_out, bn0g_in, bn0b_in, s0tab, t0tab, 0, R)

        def load_conv_w(pool, src, t):
            w = pool.tile([CZ, t, 128], BF16, tag="cw")
            nc.sync.dma_start(out=w[:], in_=src.rearrange("t ci co -> ci t co"))
            return w

        # ============ Phase C1: b0 conv1 (dil 48) ============
        with tc.tile_pool(name="c1w", bufs=1) as c1w, \
             tc.tile_pool(name="c1win", bufs=1) as c1win, \
             tc.tile_pool(name="c1t", bufs=4) as c1t, \
             tc.tile_pool(name="c1p", bufs=6, space="PSUM") as c1p:
            wt1 = load_conv_w(c1w, cw0a_in, 9)

            def load_x0(r, tag):
                raw = c1t.tile([128, L], BF16, tag="raw")
                nc.sync.dma_start(out=raw[:], in_=xpre_d[:, r, :])
                w = c1win.tile([128, L], BF16, tag=tag)
                nc.scalar.activation(
                    out=w[:], in_=raw[:], func=AF.Relu,
                    scale=s0tab[:, r : r + 1], bias=t0tab[:, r : r + 1],
                )
                return w

            def post_c1(l, psum):
                o = c1t.tile([128, L], BF16, tag="out")
                nc.scalar.activation(
                    out=o[:], in_=psum[:], func=AF.Relu, scale=mask[:, l : l + 1]
                )
                nc.sync.dma_start(out=b0c1r_d[:, l - L_B0C1_LO, :], in_=o[:])

            _conv_rows(nc, c1win, c1p, dict(
                wt=wt1, taps=_taps(3), dil=48, load_row=load_x0,
                out_range=(L_B0C1_LO, L_B0C1_HI), in_n=R, off=0,
                post=post_c1, nwin=100,
            ))

        # ============ Phase C2: b0 conv2 (dil 1) + identity -> x1 =========
        with tc.tile_pool(name="c2w", bufs=1) as c2w, \
             tc.tile_pool(name="c2win", bufs=1) as c2win, \
             tc.tile_pool(name="c2t", bufs=4) as c2t, \
             tc.tile_pool(name="c2b", bufs=1) as c2b, \
             tc.tile_pool(name="c2p", bufs=6, space="PSUM") as c2p:
            wt2 = load_conv_w(c2w, cw0b_in, 9)
            bn1s = c2b.tile([128, N_X1], F32, tag="bn1s")
            bn1q = c2b.tile([128, OWN], F32, tag="bn1q")

            def load_c1r(r, tag):
                w = c2win.tile([128, L], BF16, tag=tag)
                nc.sync.dma_start(out=w[:], in_=b0c1r_d[:, r, :])
                return w

            def post_c2(l, psum):
                xr = c2t.tile([128, L], BF16, tag="xprer")
                nc.sync.dma_start(out=xr[:], in_=xpre_d[:, l, :])
                x1r = c2t.tile([128, L], BF16, tag="x1r")
                nc.vector.scalar_tensor_tensor(
                    out=x1r[:], in0=psum[:], scalar=1.0, in1=xr[:],
                    op0=ALU.mult, op1=ALU.add,
                    accum_out=bn1s[:, l - L_X1_LO : l - L_X1_LO + 1],
                )
                nc.sync.dma_start(out=x1_d[:, l - L_X1_LO, :], in_=x1r[:])
                if L_OWN_LO <= l < L_OWN_HI:
                    junk = c2t.tile([128, L], BF16, tag="junk2")
                    nc.scalar.activation(
                        out=junk[:], in_=x1r[:], func=AF.Square,
                        accum_out=bn1q[:, l - L_OWN_LO : l - L_OWN_LO + 1],
                    )

            _conv_rows(nc, c2win, c2p, dict(
                wt=wt2, taps=_taps(3), dil=1, load_row=load_c1r,
                out_range=(L_X1_LO, L_X1_HI), in_n=N_B0C1, off=L_B0C1_LO,
                post=post_c2, nwin=6,
            ))
            part1 = c2b.tile([CZ, 2], F32, tag="part1")
            nc.vector.reduce_sum(
                part1[:, 0:1],
                bn1s[:, L_OWN_LO - L_X1_LO : L_OWN_HI - L_X1_LO],
                mybir.AxisListType.X,
            )
            nc.vector.reduce_sum(part1[:, 1:2], bn1q[:], mybir.AxisListType.X)
            nc.sync.dma_start(out=ccr1_in[:], in_=part1[:])

        nc.gpsimd.collective_compute(
            "AllReduce", ALU.add,
            ins=[ccr1_in.opt()], outs=[ccr1_out.opt()],
            replica_groups=[list(range(NC))],
        )
        bn_fold(ccr1_out, bn1g_in, bn1b_in, s1tab, t1tab, L_X1_LO, L_X1_HI)

        # ============ Phase C3: b1 conv1 (dil 36) ============
        with tc.tile_pool(name="c3w", bufs=1) as c3w, \
             tc.tile_pool(name="c3win", bufs=1) as c3win, \
             tc.tile_pool(name="c3t", bufs=4) as c3t, \
             tc.tile_pool(name="c3p", bufs=6, space="PSUM") as c3p:
            wt3 = load_conv_w(c3w, cw1a_in, 9)

            def load_x1r(r, tag):
                raw = c3t.tile([128, L], BF16, tag="raw3")
                nc.sync.dma_start(out=raw[:], in_=x1_d[:, r, :])
                w = c3win.tile([128, L], BF16, tag=tag)
                nc.scalar.activation(
                    out=w[:], in_=raw[:], func=AF.Relu,
                    scale=s1tab[:, r : r + 1], bias=t1tab[:, r : r + 1],
                )
                return w

            def post_c3(l, psum):
                o = c3t.tile([128, L], BF16, tag="out3")
                nc.scalar.activation(
                    out=o[:], in_=psum[:], func=AF.Relu, scale=mask[:, l : l + 1]
                )
                nc.sync.dma_start(out=b1c1r_d[:, l - L_B1C1_LO, :], in_=o[:])

            _conv_rows(nc, c3win, c3p, dict(
                wt=wt3, taps=_taps(3), dil=36, load_row=load_x1r,
                out_range=(L_B1C1_LO, L_B1C1_HI), in_n=N_X1, off=L_X1_LO,
                post=post_c3, nwin=78,
            ))

        # ============ Phase C4: b1 conv2 (dil 2) + identity -> x2 =========
        with tc.tile_pool(name="c4w", bufs=1) as c4w, \
             tc.tile_pool(name="c4win", bufs=1) as c4win, \
             tc.tile_pool(name="c4t", bufs=4) as c4t, \
             tc.tile_pool(name="c4p", bufs=6, space="PSUM") as c4p:
            wt4 = load_conv_w(c4w, cw1b_in, 9)

            def load_c3r(r, tag):
                w = c4win.tile([128, L], BF16, tag=tag)
                nc.sync.dma_start(out=w[:], in_=b1c1r_d[:, r, :])
                return w

            def post_c4(l, psum):
                x1r = c4t.tile([128, L], BF16, tag="x1rr")
                nc.sync.dma_start(out=x1r[:], in_=x1_d[:, l - L_X1_LO, :])
                tmp = c4t.tile([128, L], F32, tag="tmp4")
                nc.vector.tensor_tensor(tmp[:], psum[:], x1r[:], ALU.add)
                x2r = c4t.tile([128, L], BF16, tag="x2r")
                nc.vector.tensor_scalar_mul(x2r[:], tmp[:], mask[:, l : l + 1])
                nc.sync.dma_start(out=x2_d[:, l - L_X2_LO, :], in_=x2r[:])

            _conv_rows(nc, c4win, c4p, dict(
                wt=wt4, taps=_taps(3), dil=2, load_row=load_c3r,
                out_range=(L_X2_LO, L_X2_HI), in_n=N_B1C1, off=L_B1C1_LO,
                post=post_c4, nwin=8,
            ))

        # ============ Phase C5: conv_out (7x7) -> pairf ============
        mlpw = ctx.enter_context(tc.tile_pool(name="mlpw", bufs=1))
        pairf = mlpw.tile([CZ, OWN, L], F32, tag="pairf")
        pairfb = mlpw.tile([CZ, OWN, L], BF16, tag="pairfb")
        with tc.tile_pool(name="c5w", bufs=1) as c5w, \
             tc.tile_pool(name="c5win", bufs=1) as c5win, \
             tc.tile_pool(name="c5p", bufs=6, space="PSUM") as c5p:
            wt5 = load_conv_w(c5w, cwo_in, 49)
            cob = c5w.tile([CZ, 1], F32, tag="cob")
            nc.sync.dma_start(out=cob[:], in_=cob_in[:])

            def load_x2(r, tag):
                w = c5win.tile([128, L], BF16, tag=tag)
                nc.sync.dma_start(out=w[:], in_=x2_d[:, r, :])
                return w

            def post_c5(l, psum):
                i = l - L_OWN_LO
                nc.scalar.activation(
                    out=pairf[:, i, :], in_=psum[:], func=AF.Identity, bias=cob[:]
                )
                nc.vector.tensor_copy(pairfb[:, i, :], pairf[:, i, :])

            _conv_rows(nc, c5win, c5p, dict(
                wt=wt5, taps=_taps(7), dil=1, load_row=load_x2,
                out_range=(L_OWN_LO, L_OWN_HI), in_n=N_X2, off=L_X2_LO,
                post=post_c5, nwin=12,
            ))

        # ============ Phase M: pair MLP (LN folded into matmul chain) =====
        NCH = OWNPIX // 512  # 36 chunks of 512 px
        pfb = pairfb.rearrange("c r l -> c (r l)")
        pff = pairf.rearrange("c r l -> c (r l)")
        with tc.tile_pool(name="mst", bufs=1) as mst, \
             tc.tile_pool(name="mtmp", bufs=3) as mtmp:
            onesc = mst.tile([CZ, 1], BF16, tag="onesc")
            nc.vector.memset(onesc[:], 1.0)
            stat_m = mst.tile([128, NCH * 4], F32, tag="statm")
            stat_q = mst.tile([128, NCH * 4], F32, tag="statq")
            with tc.tile_pool(name="msp1", bufs=4, space="PSUM") as msp1, \
                 tc.tile_pool(name="mcp", bufs=4) as mcp:
                for q in range(NCH):
                    sl = slice(512 * q, 512 * q + 512)
                    sq = mtmp.tile([CZ, 512], BF16, tag="sq")
                    nc.vector.tensor_mul(sq[:], pfb[:, sl], pfb[:, sl])
                    pm_ = msp1.tile([1, 512], F32, tag="pm")
                    nc.tensor.matmul(
                        pm_[:], onesc[:], pfb[:, sl], start=True, stop=True
                    )
                    pq_ = msp1.tile([1, 512], F32, tag="pq")
                    nc.tensor.matmul(pq_[:], onesc[:], sq[:], start=True, stop=True)
                    cm = mcp.tile([1, 512], F32, tag="cm")
                    nc.scalar.copy(cm[:], pm_[:])
                    cq = mcp.tile([1, 512], F32, tag="cq")
                    nc.scalar.copy(cq[:], pq_[:])
                    nc.sync.dma_start(out=stat_m[:, 4 * q : 4 * q + 4], in_=cm[:])
                    nc.sync.dma_start(out=stat_q[:, 4 * q : 4 * q + 4], in_=cq[:])
            # rstd and mean*rstd in [128, NCH*4] (px = 512*q + 4*p + f)
            nc.vector.tensor_scalar_mul(stat_m[:], stat_m[:], 1.0 / CZ)
            nc.vector.tensor_scalar_mul(stat_q[:], stat_q[:], 1.0 / CZ)
            var = mst.tile([128, NCH * 4], F32, tag="varp")
            nc.vector.tensor_mul(var[:], stat_m[:], stat_m[:])
            nc.vector.tensor_sub(var[:], stat_q[:], var[:])
            nc.vector.tensor_scalar_add(var[:], var[:], EPS)
            rstd = mst.tile([128, NCH * 4], F32, tag="rstdp")
            nc.scalar.activation(out=rstd[:], in_=var[:], func=AF.Sqrt)
            nc.vector.reciprocal(rstd[:], rstd[:])
            mrs = mst.tile([128, NCH * 4], F32, tag="mrsp")
            nc.vector.tensor_mul(mrs[:], stat_m[:], rstd[:])
            rstdb = mst.tile([128, NCH * 4], BF16, tag="rstdb")
            nc.vector.tensor_copy(rstdb[:], rstd[:])
            mrsb = mst.tile([128, NCH * 4], BF16, tag="mrsb")
            nc.vector.tensor_copy(mrsb[:], mrs[:])

            w1p = mst.tile([CZ, 4 * CZ], BF16, tag="w1p")
            nc.sync.dma_start(out=w1p[:], in_=w1p_in[:])
            w2p = mst.tile([128, 4, CZ], BF16, tag="w2p")
            nc.sync.dma_start(
                out=w2p[:], in_=w2p_in.rearrange("(a p) n -> p a n", p=128)
            )
            b1pr = mst.tile([1, 4 * CZ], BF16, tag="b1pr")
            nc.sync.dma_start(out=b1pr[:], in_=b1p_in[:])
            ncs1 = mst.tile([1, 4 * CZ], BF16, tag="ncs1")
            nc.sync.dma_start(out=ncs1[:], in_=ncs1_in[:])
            b2p = mst.tile([CZ, 1], F32, tag="b2p")
            nc.sync.dma_start(out=b2p[:], in_=b2p_in[:])
            onesp = mst.tile([1, 128], BF16, tag="onesp")
            nc.vector.memset(onesp[:], 1.0)

            pout = pair_out.rearrange("c r l -> c (r l)")
            with tc.tile_pool(name="msp2", bufs=2, space="PSUM") as msp2:
                for q in range(NCH):
                    sl = slice(512 * q, 512 * q + 512)
                    rs_c = mtmp.tile([1, 512], BF16, tag="rs_c")
                    nc.sync.dma_start(
                        out=rs_c[:], in_=rstdb[:, 4 * q : 4 * q + 4]
                    )
                    mr_c = mtmp.tile([1, 512], BF16, tag="mr_c")
                    nc.sync.dma_start(
                        out=mr_c[:], in_=mrsb[:, 4 * q : 4 * q + 4]
                    )
                    prb = msp2.tile([128, 512], F32, tag="prb")
                    nc.tensor.matmul(
                        prb[:], onesp[:], rs_c[:], start=True, stop=True
                    )
                    rb = mtmp.tile([128, 512], BF16, tag="rb")
                    nc.vector.tensor_copy(rb[:], prb[:])
                    xs = mtmp.tile([128, 512], BF16, tag="xs")
                    nc.vector.tensor_mul(xs[:], pfb[:, sl], rb[:])
                    h1r_t = mtmp.tile([128, 4, 512], BF16, tag="h1rp")
                    for j in range(4):
                        jsl = slice(128 * j, 128 * j + 128)
                        p1 = msp2.tile([128, 512], F32, tag=f"p1_{j % 2}", bufs=1)
                        nc.tensor.matmul(
                            p1[:], w1p[:, jsl], xs[:], start=True, stop=False
                        )
                        nc.tensor.matmul(
                            p1[:], ncs1[:, jsl], mr_c[:], start=False, stop=False
                        )
                        nc.tensor.matmul(
                            p1[:], b1pr[:, jsl], ones_row[:], start=False, stop=True
                        )
                        nc.scalar.activation(
                            out=h1r_t[:, j, :], in_=p1[:], func=AF.Relu
                        )
                    p2 = msp2.tile([128, 512], F32, tag="p2")
                    for j in range(4):
                        nc.tensor.matmul(
                            p2[:], w2p[:, j, :], h1r_t[:, j, :],
                            start=(j == 0), stop=(j == 3),
                        )
                    outc = mtmp.tile([128, 512], F32, tag="outc")
                    nc.vector.scalar_tensor_tensor(
                        out=outc[:], in0=p2[:], scalar=b2p[:], in1=pff[:, sl],
                        op0=ALU.add, op1=ALU.add,
                    )
                    nc.sync.dma_start(out=pout[:, sl], in_=outc[:])

    nc.finalize()
    return nc


def _prep_inputs(i):
    """Host-side: weight folds, layout transforms, shards. Returns in_maps."""
    f32 = np.float32
    g0v, b0v = np.asarray(i["ln0_g"], f32), np.asarray(i["ln0_b"], f32)
    qkv = np.asarray(i["attn_qkv_w"], f32).reshape(CS, H, 3, HW)
    Wq = qkv[:, :, 0, :].reshape(CS, CS) * (HW ** -0.5)
    Wk = qkv[:, :, 1, :].reshape(CS, CS)
    Wv = qkv[:, :, 2, :].reshape(CS, CS)

    def fold(w):
        return g0v[:, None] * w, b0v @ w

    Wqf, bq = fold(Wq)
    Wkf, bk = fold(Wk)
    Wvf, bv = fold(Wv)
    Wgf = g0v[:, None] * np.asarray(i["attn_g_w"], f32)
    bg = b0v @ np.asarray(i["attn_g_w"], f32) + np.asarray(i["attn_g_b"], f32)
    Wo = np.asarray(i["attn_o_w"], f32)
    bo = np.asarray(i["attn_o_b"], f32)

    gs, bs = np.asarray(i["mlps_ln_g"], f32), np.asarray(i["mlps_ln_b"], f32)
    W1s = gs[:, None] * np.asarray(i["mlps_w1"], f32)
    b1s = bs @ np.asarray(i["mlps_w1"], f32) + np.asarray(i["mlps_b1"], f32)
    W2s = np.asarray(i["mlps_w2"], f32)
    b2s = np.asarray(i["mlps_b2"], f32)

    gp_, bp_ = np.asarray(i["s2p_ln_g"], f32), np.asarray(i["s2p_ln_b"], f32)
    Wsp = gp_[:, None] * np.asarray(i["s2p_proj_w"], f32)
    bsp = bp_ @ np.asarray(i["s2p_proj_w"], f32) + np.asarray(i["s2p_proj_b"], f32)
    Wso = np.asarray(i["s2p_o_w"], f32)
    bso = np.asarray(i["s2p_o_b"], f32)

    g2, b2 = np.asarray(i["p2s_ln_g"], f32), np.asarray(i["p2s_ln_b"], f32)
    Wp2s = g2[:, None] * np.asarray(i["p2s_w"], f32)
    bp2s = b2 @ np.asarray(i["p2s_w"], f32)

    gm, bm = np.asarray(i["mlpp_ln_g"], f32), np.asarray(i["mlpp_ln_b"], f32)
    W1p = gm[:, None] * np.asarray(i["mlpp_w1"], f32)
    b1p = bm @ np.asarray(i["mlpp_w1"], f32) + np.asarray(i["mlpp_b1"], f32)
    ncs1 = -W1p.sum(0)
    W2p = np.asarray(i["mlpp_w2"], f32)
    b2p = np.asarray(i["mlpp_b2"], f32)

    def conv_taps(w, kk):
        w = np.asarray(w, f32)
        return np.stack([w[ky, kx] for ky, kx in _taps(kk)], 0)

    bf = ml_dtypes.bfloat16
    seq = np.asarray(i["seq"], f32).reshape(L, CS)
    pair = np.asarray(i["pair"], f32).reshape(L, L, CZ)

    shared = dict(
        seq_full=seq,
        ident_in=np.eye(128, dtype=bf),
        wq=Wqf.astype(bf), wk=Wkf.astype(bf), wv=Wvf.astype(bf),
        wg=Wgf.astype(bf), wo=Wo.astype(bf),
        w1s=W1s.astype(bf), w2s=W2s.astype(bf),
        wsp=Wsp.astype(bf), wso=Wso.astype(bf), wp2s=Wp2s.astype(bf),
        w1p=W1p.astype(bf), w2p=W2p.astype(bf),
        cw0a=conv_taps(i["rb0_c1_w"], 3).astype(bf),
        cw0b=conv_taps(i["rb0_c2_w"], 3).astype(bf),
        cw1a=conv_taps(i["rb1_c1_w"], 3).astype(bf),
        cw1b=conv_taps(i["rb1_c2_w"], 3).astype(bf),
        cwo=conv_taps(i["conv_out_w"], 7).astype(bf),
        bq=np.ascontiguousarray(bq.reshape(H, HW).T).astype(f32),
        bk=np.ascontiguousarray(bk.reshape(H, HW).T).astype(f32),
        bv=bv.reshape(1, CS).astype(bf), bg=bg.reshape(1, CS).astype(bf),
        bo=bo.reshape(1, CS).astype(bf),
        b1s=b1s.reshape(1, -1).astype(bf), b2s=b2s.reshape(1, -1).astype(bf),
        bsp=bsp.reshape(1, -1).astype(bf),
        bso=bso.reshape(CZ, 1).astype(f32),
        bp2s=bp2s.reshape(1, -1).astype(bf),
        b1p=b1p.reshape(1, -1).astype(bf),
        ncs1=ncs1.reshape(1, -1).astype(bf),
        b2p=b2p.reshape(CZ, 1).astype(f32),
        cob=np.asarray(i["conv_out_b"], f32).reshape(CZ, 1),
        bn0g=np.asarray(i["rb0_bn_g"], f32).reshape(CZ, 1),
        bn0b=np.asarray(i["rb0_bn_b"], f32).reshape(CZ, 1),
        bn1g=np.asarray(i["rb1_bn_g"], f32).reshape(CZ, 1),
        bn1b=np.asarray(i["rb1_bn_b"], f32).reshape(CZ, 1),
    )

    in_maps = []
    for k in range(NC):
        g0 = OWN * k - HALO
        gs_ = np.arange(g0, g0 + R)
        valid = (gs_ >= 0) & (gs_ < L)
        psh = np.zeros((R, L, CZ), f32)
        psh[valid] = pair[gs_[valid]]
        mask_k = np.ascontiguousarray(
            np.broadcast_to(valid.astype(f32)[None, :], (128, R))
        )
        sel = np.zeros((L, R), f32)
        sel[gs_[valid], np.nonzero(valid)[0]] = 1.0
        m = dict(shared)
        m.update(
            pair_cT=np.ascontiguousarray(psh.transpose(2, 0, 1)).astype(bf),
            pair_px=np.ascontiguousarray(
                pair[OWN * k : OWN * k + OWN].reshape(-1, CZ)
            ),
            seq_own=np.ascontiguousarray(seq[OWN * k : OWN * k + OWN]),
            mask_in=mask_k,
            sel_in=sel.astype(bf),
        )
        in_maps.append(m)
    return in_maps


def kernel(**inputs):
    if "nc" not in _CACHE:
        _CACHE["nc"] = _build_program()
    nc = _CACHE["nc"]
    in_maps = _prep_inputs(inputs)
    res = run_bass_kernel_spmd(nc, in_maps, list(range(NC)))
    _CACHE["last_res"] = res
    seq_full = np.concatenate(
        [np.asarray(res.results[c]["seq_out"], np.float32) for c in range(NC)], 0
    ).reshape(B, L, CS)
    pair_full = np.concatenate(
        [
            np.asarray(res.results[c]["pair_out"], np.float32).transpose(1, 2, 0)
            for c in range(NC)
        ],
        0,
    ).reshape(B, L, L, CZ)
    return seq_full, pair_full
